# revision 41
# baseline (speedup 1.0000x reference)
"""Trainium2 Bass kernel for LongcatFlash MoE experts (expert-parallel, 8 cores).

Problem: T=4096 tokens, H=1024, I=512, 32 routed + 8 zero (identity) experts,
top-4 routing, per-expert capacity 768.

Strategy (sharding_hint = expert parallelism):
  - Host: compute routing (stable sort by expert, capacity clip), permute
    tokens to their expert's core (the "all-to-all"), build per-core packed
    activation buffers with tokens on the GEMM free dimension.
  - Device (8 cores, SPMD): each core owns 4 routed experts; per expert run
    the gated MLP as fp8(e4m3) DoubleRow matmuls (K=256 contraction per
    instruction, 2x bf16 PE throughput, half the HBM bytes):
        gu[o, c]  = sum_h guT[h, o] * xT[h, c]      (o = 2I rows, c = tokens)
        mid[i, c] = silu(gate[i, c]) * up[i, c]     (fp32 silu, fp8 mid)
        y[h, c]   = sum_i dnT[i, h] * mid[i, c]     (bf16 out)
    Accumulation is fp32 in PSUM; only operand storage is fp8. Measured
    end-to-end max rel err ~7e-3 (threshold 2e-2).
  - Warmup matmuls on scratch SBUF keep the PE HAM clock-gate at 8/8 while
    expert 0's weights stream in; expert 0's gu/x/dn DMAs are split into
    ~128KB stripes spread over the three DMA-capable queues (sync/scalar/
    gpsimd) so the first matmul starts as soon as its own stripe lands.
  - Host: gather per-assignment outputs, scale by router weight, scatter-add
    back per token, add the zero-expert weighted-identity term.
"""

import os

import numpy as np

N_CORES = 8
R = 32  # routed experts
E_PER_CORE = R // N_CORES  # 4
CAPACITY = 768
H = 1024
I_DIM = 512
HT = H // 128  # 8 h-tiles
HP = HT // 2  # 4 h-pairs (DoubleRow K=256)
IT = I_DIM // 128  # 4 i-tiles
IP = IT // 2  # 2 i-pairs
OG = I_DIM // 128  # 4 o-tiles each for gate and up

N_WARM = int(os.environ.get("MOE_WARM", "16"))

LAST_RUN = {}  # filled with exec_time_ns etc. for test harness use


def _route(idx, wts, n_tok):
    """Replicates the reference's capacity-buffer routing exactly.

    Returns per-assignment (expert, token, weight, slot, flat_index) for kept
    routed assignments, sorted by expert (stable), plus zero-expert weights.
    """
    K = idx.shape[1]
    A = n_tok * K
    flat_e = idx.reshape(-1).astype(np.int64)
    flat_t = np.repeat(np.arange(n_tok, dtype=np.int64), K)
    flat_w = wts.reshape(-1)
    order = np.argsort(flat_e, kind="stable")
    se = flat_e[order]
    st = flat_t[order]
    sw = flat_w[order]
    counts = np.bincount(flat_e, minlength=R + 8)
    starts = np.cumsum(counts) - counts
    pos = np.arange(A, dtype=np.int64) - starts[se]
    valid = (se < R) & (pos < CAPACITY)
    zero_w = np.where(idx >= R, wts, 0.0).sum(axis=1)
    return (
        se[valid],
        st[valid],
        sw[valid],
        pos[valid],
        order[valid],
        zero_w,
    )


def _chunks(S):
    # chunk the token free-dim at 512 (PSUM bank limit); S is a multiple of 64
    out = []
    c0 = 0
    while c0 < S:
        cn = min(512, S - c0)
        out.append((c0, cn))
        c0 += cn
    return out


_BUILD_CACHE = {}


def _build_bass(S):
    import concourse.bacc as bacc
    import concourse.mybir as mybir
    from concourse import tile

    if S in _BUILD_CACHE:
        return _BUILD_CACHE[S]

    FT = mybir.dt.float32
    F8 = mybir.dt.float8e4
    BF = mybir.dt.bfloat16
    DR = mybir.MatmulPerfMode.DoubleRow

    chunks = _chunks(S)

    nc = bacc.Bacc(None)
    xt_d = nc.declare_dram_parameter("xt", [E_PER_CORE, 128, HP, 2, S], F8, isOutput=False)
    # gate/up halves split at dim 2 so expert 0 can stream ~128KB stripes
    gu_d = nc.declare_dram_parameter("guw", [E_PER_CORE, 128, HP, 2, 2, 512], F8, isOutput=False)
    dn_d = nc.declare_dram_parameter("dnw", [E_PER_CORE, 128, IP, 2, 1024], F8, isOutput=False)
    # y in fp8: halves HBM-out traffic; adds ~1.4e-3 to max rel err
    yt_d = nc.declare_dram_parameter("yt", [E_PER_CORE, 128, HT, S], F8, isOutput=True)

    silu_fn = mybir.ActivationFunctionType.Silu
    copy_fn = mybir.ActivationFunctionType.Copy

    with tile.TileContext(nc) as tc:
        with (
            tc.tile_pool(name="wpool", bufs=2) as wpool,
            tc.tile_pool(name="xpool", bufs=2) as xpool,
            tc.tile_pool(name="gupool", bufs=2) as gupool,
            tc.tile_pool(name="dnpool", bufs=2) as dnpool,
            tc.tile_pool(name="midpool", bufs=2 * IP * len(chunks)) as midpool,
            # sil tiles are ACT-written; unique slots (no reuse) keep the
            # Activation instruction at a single sync-wait (AC struct limit 1)
            tc.tile_pool(name="silpool", bufs=E_PER_CORE * OG * len(chunks)) as silpool,
            tc.tile_pool(name="ypool", bufs=2) as ypool,
            tc.tile_pool(name="pgpool", bufs=2, space="PSUM") as pgpool,
            tc.tile_pool(name="pupool", bufs=2, space="PSUM") as pupool,
            tc.tile_pool(name="pypool", bufs=2, space="PSUM") as pypool,
        ):
            # ---- PE warmup: dummy DoubleRow matmuls on (uninitialized)
            # scratch SBUF keep the HAM activity monitor busy (K=8/8) while
            # expert 0's weights load. pw is never read; garbage is fine.
            wsrc = wpool.tile([128, 2, 64], F8, tag="wsrc")
            wmov = wpool.tile([128, 2, 256], F8, tag="wmov")
            nc.gpsimd.memset(wsrc[:], 0)
            nc.gpsimd.memset(wmov[:], 0)
            pw = pypool.tile([128, 2, 512], FT, tag="py")
            for k in range(N_WARM):
                nc.tensor.matmul(
                    pw[0:64, k % 2, 0:256], wsrc[:], wmov[:],
                    start=True, stop=True, perf_mode=DR,
                )

            # per-ring load plan: need-ordered FIFOs. sync/scalar = HWDGE,
            # gpsimd = SWDGE. y0-y2: one whole-expert flush each (big burst,
            # one completion stall); y3: per-hpair, alternating HWDGE rings.
            # y0 on scalar: keeps the sync ring free for gu1/x3, whose
            # need-times are tight; scalar has mid-run slack
            yq_map = {0: "scalar", 1: "scalar", 2: "sync",
                      3: ["sync", "scalar", "sync", "scalar"]}
            tiles = {}

            def _load_expert(e):
                xt = xpool.tile([128, HP, 2, S], F8, tag="xt", name=f"xt{e}")
                gut = gupool.tile([128, HP, 2, 2, 512], F8, tag="gu", name=f"gut{e}")
                dnt = dnpool.tile([128, IP, 2, 1024], F8, tag="dn", name=f"dnt{e}")
                if e == 0:
                    # stream expert 0 in ~128KB stripes across all 3 queues;
                    # the first matmul only waits for x-hp0 + gate-hp0
                    for hp in (0, 2, 1, 3):
                        nc.gpsimd.dma_start(xt[:, hp], xt_d[e, :, hp])
                    for g in range(2):
                        for hp in range(HP):
                            gq = nc.sync if hp % 2 == 0 else nc.scalar
                            gq.dma_start(gut[:, hp, g], gu_d[e, :, hp, g])
                    nc.scalar.dma_start(dnt[:], dn_d[e])
                elif e == 1:
                    nc.scalar.dma_start(xt[:], xt_d[e])
                    nc.sync.dma_start(gut[:], gu_d[e])
                    nc.scalar.dma_start(dnt[:], dn_d[e])
                elif e == 2:
                    nc.gpsimd.dma_start(xt[:], xt_d[e])
                    nc.gpsimd.dma_start(gut[:], gu_d[e])
                    nc.gpsimd.dma_start(dnt[:], dn_d[e])
                else:
                    nc.sync.dma_start(xt[:], xt_d[e])
                    nc.gpsimd.dma_start(gut[:], gu_d[e])
                    nc.scalar.dma_start(dnt[:], dn_d[e])
                tiles[e] = (xt, gut, dnt)

            _load_expert(0)
            for e in range(E_PER_CORE):
                xt, gut, dnt = tiles.pop(e)
                ywide = ypool.tile([128, HT, S], F8, tag="yo")
                for ci, (c0, cn) in enumerate(chunks):
                    mids = [
                        midpool.tile([128, 2, cn], F8, tag="mid", name=f"mid{e}_{ci}_{q}")
                        for q in range(IP)
                    ]
                    for oi in range(OG):
                        pg = pgpool.tile([128, cn], FT, tag="pg")
                        pu = pupool.tile([128, cn], FT, tag="pu")
                        # hp order (0,2,1,3): consume the sync-ring stripes
                        # first — the scalar ring's first stripes trail by
                        # ~1.3us behind its ACT table load (PSUM accumulation
                        # order doesn't matter)
                        for idx, hp in enumerate((0, 2, 1, 3)):
                            nc.tensor.matmul(
                                pg[:],
                                gut[:, hp, 0, :, oi * 128 : (oi + 1) * 128],
                                xt[:, hp, :, c0 : c0 + cn],
                                start=(idx == 0),
                                stop=(idx == HP - 1),
                                perf_mode=DR,
                            )
                        for idx, hp in enumerate((0, 2, 1, 3)):
                            nc.tensor.matmul(
                                pu[:],
                                gut[:, hp, 1, :, oi * 128 : (oi + 1) * 128],
                                xt[:, hp, :, c0 : c0 + cn],
                                start=(idx == 0),
                                stop=(idx == HP - 1),
                                perf_mode=DR,
                            )
                        sil = silpool.tile([128, cn], FT, tag="sil")
                        nc.scalar.activation(sil[:], pg[:], silu_fn)
                        q, j = divmod(oi, 2)
                        nc.vector.scalar_tensor_tensor(
                            mids[q][:, j], pu[:], 1.0, sil[:],
                            mybir.AluOpType.mult, mybir.AluOpType.mult,
                        )
                    # prefetch the next expert's tensors now, BEFORE this
                    # expert's y flushes are queued, so the weight DMAs sit
                    # ahead of the output traffic in each ring's FIFO
                    if ci == 0 and e + 1 < E_PER_CORE and e + 1 not in tiles:
                        _load_expert(e + 1)
                    # down GEMM in waves of 2 h-pairs: all q0 contractions
                    # first, so the PE has ~0.8us of q0 work in flight while
                    # the last mid (q1, from oi=3's silu+mult) lands
                    for wave in range(HT // 4):
                        hpairs = (2 * wave, 2 * wave + 1)
                        pys = {}
                        for q in range(IP):
                            for hpair in hpairs:
                                if q == 0:
                                    pys[hpair] = pypool.tile(
                                        [128, 2, 512], FT, tag="py",
                                        name=f"py{e}_{ci}_{hpair}",
                                    )
                                for k in range(2):
                                    ht = 2 * hpair + k
                                    nc.tensor.matmul(
                                        pys[hpair][:, k, 0:cn],
                                        dnt[:, q, :, ht * 128 : (ht + 1) * 128],
                                        mids[q][:],
                                        start=(q == 0),
                                        stop=(q == IP - 1),
                                        perf_mode=DR,
                                    )
                        for hpair in hpairs:
                            py = pys[hpair]
                            # copy the two stripes on different engines in
                            # parallel: halves the PSUM-bank WAR latency the
                            # next wave's q0 matmuls see
                            for k in range(2):
                                ht = 2 * hpair + k
                                dst = ywide[:, ht, c0 : c0 + cn]
                                if k == 0:
                                    nc.vector.tensor_copy(dst, py[:, k, 0:cn])
                                else:
                                    nc.scalar.activation(dst, py[:, k, 0:cn], copy_fn)
                            if ci == len(chunks) - 1 and e == E_PER_CORE - 1:
                                if hpair < 2:
                                    yq = getattr(nc, yq_map[e][hpair])
                                    yq.dma_start(
                                        yt_d[e, :, 2 * hpair : 2 * hpair + 2],
                                        ywide[:, 2 * hpair : 2 * hpair + 2, :],
                                    )
                                else:
                                    # final stripes flush individually on
                                    # both HWDGE rings in parallel
                                    for k in range(2):
                                        ht = 2 * hpair + k
                                        yq = nc.sync if k == 0 else nc.scalar
                                        yq.dma_start(
                                            yt_d[e, :, ht : ht + 1],
                                            ywide[:, ht : ht + 1, :],
                                        )
                if e < E_PER_CORE - 1:
                    getattr(nc, yq_map[e]).dma_start(yt_d[e], ywide[:])

    nc.finalize()
    _BUILD_CACHE[S] = nc
    return nc


def _install_trace_shims():
    """Make trace=True usable in this image: provide the NTFF hook module and
    neutralize the artifact upload (no bucket access needed for local use)."""
    import sys
    import types

    try:
        import antenv.axon_hooks  # noqa: F401
    except ImportError:
        hook = None
        try:
            from trn_agent_boot.trn_boot import _ntff_profile_via_ctypes

            hook = _ntff_profile_via_ctypes("/opt/axon/libaxon_pjrt.so")
        except Exception:
            hook = None
        mod = types.ModuleType("antenv.axon_hooks")
        mod._hook = hook
        mod.get_axon_ntff_profile_hook = lambda: mod._hook
        mod.set_axon_ntff_profile_hook = lambda h: setattr(mod, "_hook", h)
        sys.modules["antenv.axon_hooks"] = mod

    import concourse.bass_utils as bu

    orig_upload = bu.upload_artifacts

    def safe_upload(tmpdir):
        try:
            return orig_upload(tmpdir)
        except Exception:
            return tmpdir

    bu.upload_artifacts = safe_upload


def kernel(**inputs):
    import ml_dtypes
    from concourse.bass_utils import run_bass_kernel_spmd

    f8 = ml_dtypes.float8_e4m3

    hidden = np.ascontiguousarray(np.asarray(inputs["hidden_states"], dtype=np.float32))
    idx = np.asarray(inputs["top_k_index"]).astype(np.int64)
    wts = np.asarray(inputs["top_k_weights"], dtype=np.float32)
    gup = np.asarray(inputs["gate_up_proj"], dtype=np.float32)
    dnp = np.asarray(inputs["down_proj"], dtype=np.float32)

    n_tok = hidden.shape[0]
    K = idx.shape[1]

    ve, vt, vw, vp, va, zero_w = _route(idx, wts, n_tok)
    cnts = np.bincount(ve, minlength=R)
    maxc = int(cnts.max())
    S = max(256, ((maxc + 63) // 64) * 64)

    estarts = np.cumsum(cnts) - cnts

    in_maps = []
    for c in range(N_CORES):
        xt = np.zeros((E_PER_CORE, 128, HT, S), dtype=f8)
        for le in range(E_PER_CORE):
            ge = c * E_PER_CORE + le
            s0, cnt = estarts[ge], cnts[ge]
            if cnt == 0:
                continue
            toks = vt[s0 : s0 + cnt]
            # [cnt, H] -> [H, cnt] -> [HT, 128, cnt] -> [128, HT, cnt]
            xbuf = hidden[toks].T.reshape(HT, 128, cnt).transpose(1, 0, 2)
            xt[le, :, :, :cnt] = xbuf.astype(f8)
        xt = xt.reshape(E_PER_CORE, 128, HP, 2, S)
        guw = (
            gup[c * E_PER_CORE : (c + 1) * E_PER_CORE]
            .transpose(0, 2, 1)  # [4, H, 2I]  (h, o) with o = [gate | up]
            .reshape(E_PER_CORE, HP, 2, 128, 2, 512)  # [e, hp, j, p, g, o']
            .transpose(0, 3, 1, 4, 2, 5)  # [e, 128, HP, g, j, o']
            .astype(f8)
        )
        dnw = (
            dnp[c * E_PER_CORE : (c + 1) * E_PER_CORE]
            .transpose(0, 2, 1)  # [4, I, H]
            .reshape(E_PER_CORE, IT, 128, 1024)
            .transpose(0, 2, 1, 3)  # [4, 128, IT, 1024]
            .reshape(E_PER_CORE, 128, IP, 2, 1024)
            .astype(f8)
        )
        in_maps.append({"xt": np.ascontiguousarray(xt),
                        "guw": np.ascontiguousarray(guw),
                        "dnw": np.ascontiguousarray(dnw)})

    nc = _build_bass(S)

    trace = bool(int(os.environ.get("KERNEL_TRACE", "0")))
    if trace:
        _install_trace_shims()
    res = run_bass_kernel_spmd(nc, in_maps, list(range(N_CORES)), trace=trace)
    LAST_RUN["exec_time_ns"] = res.exec_time_ns
    LAST_RUN["mean_exec_time_ns"] = res.mean_exec_time_ns
    LAST_RUN["instructions_and_trace"] = res.instructions_and_trace
    LAST_RUN["profile_json"] = res.profile_json

    # ---- combine on host ----
    out = hidden * zero_w[:, None].astype(np.float32)
    acc = np.zeros((n_tok * K, H), dtype=np.float32)
    for c in range(N_CORES):
        yt = np.asarray(res.results[c]["yt"]).astype(np.float32)  # [4,128,HT,S] fp8
        for le in range(E_PER_CORE):
            ge = c * E_PER_CORE + le
            s0, cnt = estarts[ge], cnts[ge]
            if cnt == 0:
                continue
            # [128, HT, S] -> [HT, 128, S] -> [H, S]
            y = yt[le].transpose(1, 0, 2).reshape(H, S)[:, :cnt].T
            acc[va[s0 : s0 + cnt]] = y * vw[s0 : s0 + cnt, None]
    out += acc.reshape(n_tok, K, H).sum(axis=1)
    return out
